# revision 45
# baseline (speedup 1.0000x reference)
"""Trainium2 Bass kernel for the IMU preintegration module.

Full inputs in, full outputs out; internally data-parallel over 8 NeuronCores
(512 batch rows per core).

Math: the scan's per-row state composes associatively as (R, b, d) with
    b = sum_t R_{1..t} a_t,   d = sum_t (S-1-t) R_{1..t} a_t   (raw units;
physical factors of dt are folded into scalars).
  L0: groups of n0=4 steps reduce to (theta, b, rho) with first-order
      rotation.  theta = s01+s23 (+ (dt/2) s01 x s23 BCH term) where
      s01/s23 are the step-pair sums of w.  The b correction
      (dt/4)(theta x u + p x aw) == (dt/2)[s01 x m1 + s23 x m2] with
      m1 = 2V - A0, m2 = A2 + 2 A3 (V = 4-step a-sum, Ai per-phase a).
      rho = 3 A0 + 2 A1 + A2.
  SA: small-angle composition up to 256-step units:
      theta' = tA+tB + (dt/2) tA x tB,  b' = bA+bB + dt (tA x bB),
      rho' = rA + n bA + rB.
  QT: convert theta -> quaternion (2-3 term polys in h = |theta dt/2|^2),
      exact quaternion binary tree for the remaining 3 levels.

Perf design (hardware-measured):
 - The host pre-permutes each slab to [4 phase][6 ch][G] blocks so every
   on-chip read is dense (strided SBUF reads waste read-port bandwidth
   and inflate concurrently-running ops on other engines 2-6x).
 - ScalarE (contention-immune) casts each slab to bf16 once; the whole
   L0 runs in bf16 2x mode on DVE.
 - The three L0 crosses are packed into one 3G-wide cross (9 ops instead
   of 27); SA-level crosses pack their operands via ScalarE copies.
 - GpSimd is idle during the slab phase: measured, its fat TT passes
   inflate concurrent DVE ops by more than the work they remove.
"""

import math
import os
import numpy as np

import concourse.mybir as mybir
from concourse import bass, bacc
from concourse.tile import TileContext

F32 = mybir.dt.float32
BF16 = mybir.dt.bfloat16
OP = mybir.AluOpType
AF = mybir.ActivationFunctionType

# problem constants (hardcoded per harness contract)
B_FULL = 4096
S_FULL = 2048
C = 6
N_CORES = 8
R = B_FULL // N_CORES          # rows per core = 512
DT = float(np.float32(1.0 / 200.0))

QSGN = [(1, -1, -1, -1), (1, 1, 1, -1), (1, -1, 1, 1), (1, 1, -1, 1)]
QIDX = [(0, 1, 2, 3), (1, 0, 3, 2), (2, 3, 0, 1), (3, 2, 1, 0)]

# step chunks per slab: short first chunk so compute starts early, short
# last chunk so the tree phase starts early
CHUNKS = [128, 384, 512, 512, 512]
assert sum(CHUNKS) == S_FULL
NCOL = S_FULL * C


def host_permute(x):
    """[B, S, 6] -> [B, S*6] with per-slab [phase(4), ch(6), G] blocks."""
    B = x.shape[0]
    outp = np.empty((B, NCOL), np.float32)
    s0 = 0
    for S_c in CHUNKS:
        G = S_c // 4
        blk = x[:, s0:s0 + S_c, :].reshape(B, G, 4, C).transpose(0, 2, 3, 1)
        outp[:, s0 * C:(s0 + S_c) * C] = blk.reshape(B, S_c * C)
        s0 += S_c
    return outp


def fuse(ap):
    # [128, g, c, n] -> [128, g*c, n] so stt sees <= 2 free dims
    return ap.rearrange("p g c n -> p (g c) n")


def build_nc(rows=R, s_len=S_FULL):
    nc = bacc.Bacc(None, target_bir_lowering=False, debug=False)
    g = rows // 128                    # 4 rows per partition
    GMAX = 128
    NU16 = s_len // 16                 # 128 16-step units per row
    x = nc.dram_tensor("x", [rows, NCOL], F32, kind="ExternalInput")
    out = nc.dram_tensor("out", [rows, 7], F32, kind="ExternalOutput")
    xv = x.rearrange("(g p) n -> g p n", g=g)

    V = nc.vector
    Gp = nc.gpsimd
    Sc = nc.scalar

    def cross_into(dst, u, v, tmps):
        """dst = u x v per-component (lists of comp views). 9 ops."""
        for i in range(3):
            i1, i2 = (i + 1) % 3, (i + 2) % 3
            V.tensor_tensor(out=tmps[0], in0=u[i1], in1=v[i2], op=OP.mult)
            V.tensor_tensor(out=tmps[1], in0=u[i2], in1=v[i1], op=OP.mult)
            V.tensor_tensor(out=dst[i], in0=tmps[0], in1=tmps[1],
                            op=OP.subtract)

    with TileContext(nc) as tc:
        with (
            tc.tile_pool(name="slab", bufs=1) as slab_pool,
            tc.tile_pool(name="slabh", bufs=2) as slabh_pool,
            tc.tile_pool(name="plane", bufs=1) as plane_pool,
            tc.tile_pool(name="pack", bufs=1) as pack_pool,
            tc.tile_pool(name="upper", bufs=1) as upper_pool,
            tc.tile_pool(name="xtmp", bufs=2) as xtmp_pool,
            tc.tile_pool(name="acc", bufs=1) as acc_pool,
            tc.tile_pool(name="tree", bufs=1) as tree_pool,
        ):
            pshape = [128, g, 3, GMAX]

            def hplane(tg, nm, n=GMAX):
                return plane_pool.tile([128, g, 3, n], BF16, tag=tg,
                                       name=nm)

            def btmp(n):
                return [xtmp_pool.tile([128, g, 3 * GMAX], BF16, tag="vt",
                                       name="vt")[:, :, 0:n]
                        for _ in range(2)]

            # persistent accumulator: [th16|b16|r16] triple at fixed
            # 128-wide segments (bf16 so SA2 writes at 2x and upper
            # level 0 reads at 2x)
            ACC = acc_pool.tile([128, g, 3, 3 * GMAX], BF16, tag="acc3",
                                name="acc3")

            def tri(ap, s=3):
                # [128,g,c,(s n)] -> [128,(g c),s,n] triple-segment view
                return ap.rearrange("p g c (s n) -> p (g c) s n", s=s)

            def sa_level(TIN, SIN, UI, TOUT, off, oth, ob, orr, n_steps,
                         cst, pk):
                """Compose adjacent units of a [th|b|r] triple tile.
                TIN: triple AP [128,g,3,3*SIN] (segments at SIN offsets,
                live width 2*UI each); TOUT: triple AP with segments at
                SOUT offsets; oth/ob/orr: the three live output segment
                views [128,g,3,UI].  One pair-add covers all three
                segments; cst gets the packed [csa|csb] cross via the
                ScalarE-packed pk tiles."""
                U = UI
                thI = TIN[:, :, :, 0:2 * U]
                bI = TIN[:, :, :, SIN:SIN + 2 * U]
                csa, csb = cst[:, :, :, 0:U], cst[:, :, :, U:2 * U]
                PL, PR = pk
                for ci in range(3):
                    Sc.copy(PL[:, :, ci, 0:U], thI[:, :, ci, 0::2])
                    Sc.copy(PL[:, :, ci, U:2 * U], thI[:, :, ci, 0::2])
                    Sc.copy(PR[:, :, ci, 0:U], thI[:, :, ci, 1::2])
                    Sc.copy(PR[:, :, ci, U:2 * U], bI[:, :, ci, 1::2])
                cross_into([cst[:, :, ci, :] for ci in range(3)],
                           [PL[:, :, ci, :] for ci in range(3)],
                           [PR[:, :, ci, :] for ci in range(3)],
                           btmp(2 * U))
                ein = tri(TIN, s=3)[:, :, :, 0:2 * U]
                V.tensor_tensor(out=tri(TOUT, s=3)[:, :, :, off:off + U],
                                in0=ein[:, :, :, 0::2],
                                in1=ein[:, :, :, 1::2], op=OP.add)
                # stt runs at 1x even in bf16, so ScalarE pre-scales the
                # fold operands (into the dead PL/PR halves) and DVE
                # folds with 2x tensor_tensor adds
                csA = PL[:, :, :, 0:U]
                csB = PL[:, :, :, U:2 * U]
                bn = PR[:, :, :, 0:U]
                Sc.activation(fuse(csA), fuse(csa), AF.Copy, scale=DT / 2.0)
                Sc.activation(fuse(csB), fuse(csb), AF.Copy, scale=DT)
                Sc.activation(fuse(bn), fuse(bI)[:, :, 0::2], AF.Copy,
                              scale=float(n_steps))
                V.tensor_tensor(out=fuse(oth), in0=fuse(csA),
                                in1=fuse(oth), op=OP.add)
                V.tensor_tensor(out=fuse(ob), in0=fuse(csB),
                                in1=fuse(ob), op=OP.add)
                V.tensor_tensor(out=fuse(orr), in0=fuse(bn),
                                in1=fuse(orr), op=OP.add)

            s0 = 0
            for si, S_c in enumerate(CHUNKS):
                G = S_c // 4
                U8 = S_c // 8
                U16 = S_c // 16
                u16o = s0 // 16

                sf = slab_pool.tile([128, g, 512 * C], F32, tag="slab",
                                    name=f"slab{si}")
                sh = slabh_pool.tile([128, g, 512 * C], BF16, tag="slabh",
                                     name=f"slabh{si}")
                # DMA phases 0-1 and 2-3 separately so the first casts
                # (and s01) start while the second half is still in
                # flight; ScalarE casts each phase block to bf16 (dense)
                H = 12 * G
                for half in range(2):
                    for gi in range(g):
                        nc.sync.dma_start(
                            out=sf[:, gi, half * H:(half + 1) * H],
                            in_=xv[gi, :, s0 * C + half * H:
                                   s0 * C + (half + 1) * H])
                for ph in range(4):
                    Sc.copy(sh[:, :, ph * 6 * G:(ph + 1) * 6 * G],
                            sf[:, :, ph * 6 * G:(ph + 1) * 6 * G])
                shv = sh[:, :, 0:S_c * C].rearrange(
                    "p g (ph ch n) -> p g ph ch n", ph=4, ch=C)
                W = [shv[:, :, i, 0:3, :] for i in range(4)]
                At = [shv[:, :, i, 3:6, :] for i in range(4)]

                # packed cross operands: Lh = [s01|s01|s23],
                # Rh = [s23|m1|m2], Ch = [c3|cr1|cr2]
                Lh = pack_pool.tile([128, g, 3, 3 * GMAX], BF16, tag="Lh",
                                    name="Lh")
                Rh = pack_pool.tile([128, g, 3, 3 * GMAX], BF16, tag="Rh",
                                    name="Rh")
                Ch = pack_pool.tile([128, g, 3, 3 * GMAX], BF16, tag="Ch",
                                    name="Ch")
                s01 = Lh[:, :, :, 0:G]
                s23 = Lh[:, :, :, 2 * G:3 * G]
                TP = hplane("tp", "tp", n=3 * GMAX)
                th4 = TP[:, :, :, 0:G]
                V.tensor_tensor(out=s01, in0=W[0], in1=W[1], op=OP.add)
                V.tensor_tensor(out=s23, in0=W[2], in1=W[3], op=OP.add)
                for ci in range(3):
                    Sc.copy(Lh[:, :, ci, G:2 * G], s01[:, :, ci, :])
                    Sc.copy(Rh[:, :, ci, 0:G], s23[:, :, ci, :])
                V.tensor_tensor(out=fuse(th4), in0=fuse(s01), in1=fuse(s23),
                                op=OP.add)

                # a-path via pair sums (all bf16 2x, 8 ops):
                #   v1 = A0+A1, v2 = A2+A3, V = v1+v2, t = A0+A2
                #   rho = 3A0+2A1+A2 = 2*v1 + t
                #   m2 = A2+2A3 = v2 + A3,  m1 = 2V-A0 = V + (V-A0)
                # th4/vv/rho4 live in one [th|b|r] triple tile (segments
                # at GMAX offsets) so SA1 pair-adds all three at once
                v1 = hplane("v1", "v1")[:, :, :, 0:G]
                v2 = hplane("rr", "v2")[:, :, :, 0:G]
                tt = hplane("t02", "t02")[:, :, :, 0:G]
                vv = TP[:, :, :, GMAX:GMAX + G]
                rho4 = TP[:, :, :, 2 * GMAX:2 * GMAX + G]
                m1h = Rh[:, :, :, G:2 * G]
                m2h = Rh[:, :, :, 2 * G:3 * G]
                V.tensor_tensor(out=v1, in0=At[0], in1=At[1], op=OP.add)
                V.tensor_tensor(out=v2, in0=At[2], in1=At[3], op=OP.add)
                V.tensor_tensor(out=fuse(vv), in0=fuse(v1), in1=fuse(v2),
                                op=OP.add)
                V.tensor_tensor(out=tt, in0=At[0], in1=At[2], op=OP.add)
                V.scalar_tensor_tensor(out=fuse(rho4), in0=fuse(v1),
                                       scalar=2.0, in1=fuse(tt),
                                       op0=OP.mult, op1=OP.add)
                V.tensor_tensor(out=m2h, in0=v2, in1=At[3], op=OP.add)
                V.tensor_tensor(out=tt, in0=vv, in1=At[0], op=OP.subtract)
                V.tensor_tensor(out=fuse(m1h), in0=fuse(vv), in1=fuse(tt),
                                op=OP.add)

                # packed crosses: [c3|cr1|cr2] = [s01|s01|s23] x
                # [s23|m1|m2] as one 3G-wide cross (9 ops, not 27)
                cross_into([Ch[:, :, ci, 0:3 * G] for ci in range(3)],
                           [Lh[:, :, ci, 0:3 * G] for ci in range(3)],
                           [Rh[:, :, ci, 0:3 * G] for ci in range(3)],
                           btmp(3 * G))
                c3h = Ch[:, :, :, 0:G]
                cr1 = Ch[:, :, :, G:2 * G]
                cr2 = Ch[:, :, :, 2 * G:3 * G]
                # folds via ScalarE prescale + 2x TT (stt is 1x-only)
                c3s = Lh[:, :, :, 0:G]          # Lh dead after the cross
                ccs = Lh[:, :, :, G:2 * G]
                Sc.activation(fuse(c3s), fuse(c3h), AF.Copy,
                              scale=DT / 2.0)
                V.tensor_tensor(out=fuse(th4), in0=fuse(c3s),
                                in1=fuse(th4), op=OP.add)
                V.tensor_tensor(out=fuse(cr1), in0=fuse(cr2),
                                in1=fuse(cr1), op=OP.add)
                Sc.activation(fuse(ccs), fuse(cr1), AF.Copy,
                              scale=DT / 2.0)
                V.tensor_tensor(out=fuse(vv), in0=fuse(ccs),
                                in1=fuse(vv), op=OP.add)

                # ---- SA1: n=4 -> 8 (triple tile, packed crosses) ----
                TP8 = hplane("tp8", "tp8", n=192)
                th8 = TP8[:, :, :, 0:U8]
                b8 = TP8[:, :, :, 64:64 + U8]
                r8 = TP8[:, :, :, 128:128 + U8]
                cs1 = hplane("cs", "cs1")
                pl1 = hplane("pl", "pl1")
                pr1 = hplane("pr", "pr1")
                sa_level(TP[:], GMAX, U8, TP8[:], 0, th8, b8, r8, 4,
                         cs1[:, :, :, 0:2 * U8],
                         (pl1[:, :, :, 0:2 * U8], pr1[:, :, :, 0:2 * U8]))

                # ---- SA2: n=8 -> 16, into the bf16 triple accumulator --
                oth = ACC[:, :, :, u16o:u16o + U16]
                ob = ACC[:, :, :, 128 + u16o:128 + u16o + U16]
                orr = ACC[:, :, :, 256 + u16o:256 + u16o + U16]
                cs2 = hplane("cs", "cs2")
                pl2 = hplane("pl", "pl2")
                pr2 = hplane("pr", "pr2")
                sa_level(TP8[:], 64, U16, ACC[:], u16o, oth, ob, orr, 8,
                         cs2[:, :, :, 0:2 * U16],
                         (pl2[:, :, :, 0:2 * U16], pr2[:, :, :, 0:2 * U16]))
                s0 += S_c

            # ---- batched SA levels: 16 -> 32 -> 64 -> 128 -> 256 ----
            # triple tiles throughout; segments at fixed 64 offsets for
            # the intermediate levels, 8 for the final 256-step triple
            cur, cseg = ACC[:], GMAX
            nu = NU16
            n_steps = 16
            for lev in range(4):
                nu //= 2
                if lev < 3:
                    a = lev % 2
                    TU = upper_pool.tile([128, g, 3, 192], F32,
                                         tag=f"u{a}", name=f"u{lev}")[:]
                    oseg = 64
                else:
                    TU = upper_pool.tile([128, g, 3, 24], F32,
                                         tag="t256", name="t256")[:]
                    oseg = 8
                cst = hplane("cs", f"ucs{lev}")
                plu = hplane("pl", f"upl{lev}")
                pru = hplane("pr", f"upr{lev}")
                sa_level(cur, cseg, nu, TU, 0,
                         TU[:, :, :, 0:nu],
                         TU[:, :, :, oseg:oseg + nu],
                         TU[:, :, :, 2 * oseg:2 * oseg + nu], n_steps,
                         cst[:, :, :, 0:2 * nu],
                         (plu[:, :, :, 0:2 * nu], pru[:, :, :, 0:2 * nu]))
                cur, cseg = TU, oseg
                n_steps *= 2

            # ---- convert 256-step units to quaternions ----
            NU = 8
            th256 = cur[:, :, :, 0:NU]
            bd256 = cur[:, :, :, NU:3 * NU].rearrange(
                "p g c (v n) -> p v g c n", v=2)
            K2 = (DT / 2.0) ** 2
            h2 = tree_pool.tile([128, g, NU], F32, tag="h2", name="h2")[:]
            hy = tree_pool.tile([128, g, NU], F32, tag="hy", name="hy")[:]
            hz = tree_pool.tile([128, g, NU], F32, tag="hz", name="hz")[:]
            q256 = tree_pool.tile([128, g, 4, NU], F32, tag="q256",
                                  name="q256")
            thc = [th256[:, :, ci, :] for ci in range(3)]
            V.tensor_tensor(out=h2, in0=thc[0], in1=thc[0], op=OP.mult)
            for ci in (1, 2):
                V.tensor_tensor(out=hy, in0=thc[ci], in1=thc[ci], op=OP.mult)
                V.tensor_tensor(out=h2, in0=h2, in1=hy, op=OP.add)
            V.scalar_tensor_tensor(out=hy, in0=h2, scalar=K2 * K2 / 24.0,
                                   in1=h2, op0=OP.mult, op1=OP.mult)
            V.scalar_tensor_tensor(out=hz, in0=h2, scalar=-K2 / 2.0,
                                   in1=hy, op0=OP.mult, op1=OP.add)
            Sc.activation(q256[:, :, 0, :], hz, AF.Identity, bias=1.0)
            V.scalar_tensor_tensor(out=hy, in0=h2, scalar=K2 * K2 / 120.0,
                                   in1=h2, op0=OP.mult, op1=OP.mult)
            V.scalar_tensor_tensor(out=hz, in0=h2, scalar=-K2 / 6.0,
                                   in1=hy, op0=OP.mult, op1=OP.add)
            Sc.activation(hz, hz, AF.Copy, scale=DT / 2.0, bias=DT / 2.0)
            for ci in range(3):
                V.tensor_tensor(out=q256[:, :, 1 + ci, :],
                                in0=thc[ci], in1=hz, op=OP.mult)

            # ---- quaternion binary tree over 8 units (3 levels) ----
            qt = q256
            bdt = bd256
            n2 = 256.0
            n_lvl = int(math.log2(NU))
            for lvl in range(1, n_lvl + 1):
                n = NU >> lvl
                nqt = tree_pool.tile([128, g, 4, n], F32, tag=f"nq{lvl}",
                                     name=f"nq{lvl}")
                nbd = tree_pool.tile([128, 2, g, 3, n], F32, tag=f"nbd{lvl}",
                                     name=f"nbd{lvl}")
                P = [tree_pool.tile([128, g, 4, n], F32, tag=f"P{a}{lvl}",
                                    name=f"P{a}")[:] for a in range(4)]
                tcc = tree_pool.tile([128, 2, g, 3, n], F32,
                                     tag=f"tcc{lvl}", name="tcc")
                tcq = tree_pool.tile([128, 2, g, 3, n], F32,
                                     tag=f"tcq{lvl}", name="tcq")
                tcw = tree_pool.tile([128, 2, g, 3, n], F32,
                                     tag=f"tcw{lvl}", name="tcw")
                tms = [tree_pool.tile([128, 2, g, n], F32,
                                      tag=f"tm{i}{lvl}", name="tms")[:]
                       for i in range(2)]

                q2all = qt[:, :, :, 1::2]
                for a in range(4):
                    q1a = qt[:, :, a, 0::2].unsqueeze(2).broadcast_to(
                        [128, g, 4, n])
                    V.tensor_tensor(out=P[a], in0=q1a, in1=q2all, op=OP.mult)
                for c in range(4):
                    idx = QIDX[c]
                    sgn = QSGN[c]
                    acc = nqt[:, :, c, :]
                    V.tensor_tensor(
                        out=acc, in0=P[0][:, :, idx[0], :],
                        in1=P[1][:, :, idx[1], :],
                        op=OP.add if sgn[1] > 0 else OP.subtract)
                    for t in (2, 3):
                        V.tensor_tensor(
                            out=acc, in0=acc, in1=P[t][:, :, idx[t], :],
                            op=OP.add if sgn[t] > 0 else OP.subtract)

                def qb(comp):
                    return qt[:, :, comp, 0::2].unsqueeze(1).broadcast_to(
                        [128, 2, g, n])

                def bd2(comp):
                    return bdt[:, :, :, comp, 1::2]

                # tcq and the nbd pair-add are independent of the serial
                # tcc->tcw chain, so GpSimd takes those two pieces.
                for i in range(3):
                    i1, i2 = (i + 1) % 3, (i + 2) % 3
                    V.tensor_tensor(out=tms[0], in0=qb(1 + i1),
                                    in1=bd2(i2), op=OP.mult)
                    V.tensor_tensor(out=tms[1], in0=qb(1 + i2),
                                    in1=bd2(i1), op=OP.mult)
                    V.tensor_tensor(out=tcc[:, :, :, i, :], in0=tms[0],
                                    in1=tms[1], op=OP.subtract)
                for i in range(3):
                    V.tensor_tensor(out=tcq[:, :, :, i, :], in0=qb(0),
                                    in1=bd2(i), op=OP.mult)
                V.tensor_tensor(out=tcc[:].opt(), in0=tcc[:].opt(),
                                in1=tcq[:].opt(), op=OP.add)
                for i in range(3):
                    i1, i2 = (i + 1) % 3, (i + 2) % 3
                    V.tensor_tensor(out=tms[0], in0=qb(1 + i1),
                                    in1=tcc[:, :, :, i2, :], op=OP.mult)
                    V.tensor_tensor(out=tms[1], in0=qb(1 + i2),
                                    in1=tcc[:, :, :, i1, :], op=OP.mult)
                    V.tensor_tensor(out=tcw[:, :, :, i, :], in0=tms[0],
                                    in1=tms[1], op=OP.subtract)
                V.tensor_tensor(out=nbd[:].opt(),
                                in0=bdt[:, :, :, :, 0::2].opt(),
                                in1=bdt[:, :, :, :, 1::2].opt(), op=OP.add)
                V.scalar_tensor_tensor(out=nbd[:].opt(), in0=tcw[:].opt(),
                                       scalar=2.0, in1=nbd[:].opt(),
                                       op0=OP.mult, op1=OP.add)
                V.scalar_tensor_tensor(
                    out=nbd[:, 1].rearrange("p g c n -> p (g c) n"),
                    in0=bdt[:, 0, :, :, 0::2].rearrange(
                        "p g c n -> p (g c) n"),
                    scalar=n2,
                    in1=nbd[:, 1].rearrange("p g c n -> p (g c) n"),
                    op0=OP.mult, op1=OP.add)

                qt, bdt = nqt, nbd
                n2 *= 2.0

            cur_q = [qt[:, :, ci, :] for ci in range(4)]
            cur_b = [bdt[:, 0, :, ci, :] for ci in range(3)]
            cur_d = [bdt[:, 1, :, ci, :] for ci in range(3)]

            # ---- finalize ----
            out_t = tree_pool.tile([128, g, 7], F32, tag="outt",
                                   name="outt")[:]
            tf = tree_pool.tile([128, g, 3], F32, tag="tf", name="tf")[:]
            for i in range(3):
                V.scalar_tensor_tensor(out=tf[:, :, i:i + 1], in0=cur_b[i],
                                       scalar=1.5, in1=cur_d[i],
                                       op0=OP.mult, op1=OP.add)
            V.tensor_scalar(out=out_t[:, :, 0:3], in0=tf,
                            scalar1=DT * DT, scalar2=None, op0=OP.mult)
            sg = tree_pool.tile([128, g, 1], F32, tag="sg", name="sg")[:]
            V.tensor_scalar(out=sg, in0=cur_q[0], scalar1=0.0, scalar2=2.0,
                            op0=OP.is_ge, op1=OP.mult)
            V.tensor_scalar(out=sg, in0=sg, scalar1=-1.0, scalar2=None,
                            op0=OP.add)
            V.tensor_tensor(out=out_t[:, :, 3:7],
                            in0=qt[:, :, :, 0],
                            in1=sg.broadcast_to([128, g, 4]), op=OP.mult)

            ov = out.rearrange("(g p) c -> p g c", g=g)
            nc.sync.dma_start(out=ov, in_=out_t)

    nc.compile()
    return nc


_NC_CACHE = {}
LAST_RESULTS = None


def _ensure_profiling_hooks():
    """Best-effort: provide the antenv.axon_hooks shim + skip S3 upload so
    trace=True works in this stripped container. No-op on failure."""
    import sys
    import types
    try:
        if "antenv.axon_hooks" not in sys.modules:
            from trn_agent_boot.trn_boot import _ntff_profile_via_ctypes
            hook = _ntff_profile_via_ctypes("/opt/axon/libaxon_pjrt.so")
            mod = types.ModuleType("antenv.axon_hooks")
            mod._hook = hook
            mod.get_axon_ntff_profile_hook = lambda: mod._hook
            mod.set_axon_ntff_profile_hook = lambda h: setattr(mod, "_hook", h)
            sys.modules["antenv.axon_hooks"] = mod
        import concourse.bass_utils as bu
        bu.upload_artifacts = lambda tmpdir: tmpdir
    except Exception as e:  # pragma: no cover
        print(f"profiling hook setup failed ({e}); tracing may be skipped")


def kernel(input_seq: np.ndarray) -> np.ndarray:
    from concourse.bass_utils import run_bass_kernel_spmd

    global LAST_RESULTS
    input_seq = np.ascontiguousarray(np.asarray(input_seq, dtype=np.float32))
    assert input_seq.shape == (B_FULL, S_FULL, C), input_seq.shape
    xp = host_permute(input_seq)

    if "nc" not in _NC_CACHE:
        _NC_CACHE["nc"] = build_nc()
    nc = _NC_CACHE["nc"]

    in_maps = [{"x": xp[i * R:(i + 1) * R]} for i in range(N_CORES)]
    trace = os.environ.get("BASS_KERNEL_TRACE", "0") == "1"
    if trace:
        _ensure_profiling_hooks()
    try:
        res = run_bass_kernel_spmd(nc, in_maps, core_ids=list(range(N_CORES)),
                                   trace=trace)
    except Exception:
        # transient device wedge (NRT_EXEC_UNIT_UNRECOVERABLE) recovers on
        # a clean re-run; retry once
        res = run_bass_kernel_spmd(nc, in_maps, core_ids=list(range(N_CORES)),
                                   trace=trace)
    LAST_RESULTS = res
    return np.concatenate([r["out"] for r in res.results], axis=0)


# revision 46
# speedup vs baseline: 1.0625x; 1.0625x over previous
"""Trainium2 Bass kernel for the IMU preintegration module.

Full inputs in, full outputs out; internally data-parallel over 8 NeuronCores
(512 batch rows per core).

Math: the scan's per-row state composes associatively as (R, b, d) with
    b = sum_t R_{1..t} a_t,   d = sum_t (S-1-t) R_{1..t} a_t   (raw units;
physical factors of dt are folded into scalars).
  L0: groups of n0=4 steps reduce to (theta, b, rho) with first-order
      rotation.  theta = s01+s23 (+ (dt/2) s01 x s23 BCH term) where
      s01/s23 are the step-pair sums of w.  The b correction
      (dt/4)(theta x u + p x aw) == (dt/2)[s01 x m1 + s23 x m2] with
      m1 = 2V - A0, m2 = A2 + 2 A3 (V = 4-step a-sum, Ai per-phase a).
      rho = 3 A0 + 2 A1 + A2.
  SA: small-angle composition up to 256-step units:
      theta' = tA+tB + (dt/2) tA x tB,  b' = bA+bB + dt (tA x bB),
      rho' = rA + n bA + rB.
  QT: convert theta -> quaternion (2-3 term polys in h = |theta dt/2|^2),
      exact quaternion binary tree for the remaining 3 levels.

Perf design (hardware-measured):
 - The host pre-permutes each slab to [4 phase][6 ch][G] blocks so every
   on-chip read is dense (strided SBUF reads waste read-port bandwidth
   and inflate concurrently-running ops on other engines 2-6x).
 - ScalarE (contention-immune) casts each slab to bf16 once; the whole
   L0 runs in bf16 2x mode on DVE.
 - The three L0 crosses are packed into one 3G-wide cross (9 ops instead
   of 27); SA-level crosses pack their operands via ScalarE copies.
 - GpSimd is idle during the slab phase: measured, its fat TT passes
   inflate concurrent DVE ops by more than the work they remove.
"""

import math
import os
import numpy as np

import concourse.mybir as mybir
from concourse import bass, bacc
from concourse.tile import TileContext

F32 = mybir.dt.float32
BF16 = mybir.dt.bfloat16
OP = mybir.AluOpType
AF = mybir.ActivationFunctionType

# problem constants (hardcoded per harness contract)
B_FULL = 4096
S_FULL = 2048
C = 6
N_CORES = 8
R = B_FULL // N_CORES          # rows per core = 512
DT = float(np.float32(1.0 / 200.0))

QSGN = [(1, -1, -1, -1), (1, 1, 1, -1), (1, -1, 1, 1), (1, 1, -1, 1)]
QIDX = [(0, 1, 2, 3), (1, 0, 3, 2), (2, 3, 0, 1), (3, 2, 1, 0)]

# step chunks per slab: short first chunk so compute starts early, short
# last chunk so the tree phase starts early
CHUNKS = [128, 384, 512, 512, 512]
assert sum(CHUNKS) == S_FULL
NCOL = S_FULL * C


def host_permute(x):
    """[B, S, 6] -> [B, S*6] with per-slab [phase(4), ch(6), G] blocks."""
    B = x.shape[0]
    outp = np.empty((B, NCOL), np.float32)
    s0 = 0
    for S_c in CHUNKS:
        G = S_c // 4
        blk = x[:, s0:s0 + S_c, :].reshape(B, G, 4, C).transpose(0, 2, 3, 1)
        outp[:, s0 * C:(s0 + S_c) * C] = blk.reshape(B, S_c * C)
        s0 += S_c
    return outp


def fuse(ap):
    # [128, g, c, n] -> [128, g*c, n] so stt sees <= 2 free dims
    return ap.rearrange("p g c n -> p (g c) n")


def build_nc(rows=R, s_len=S_FULL):
    nc = bacc.Bacc(None, target_bir_lowering=False, debug=False)
    g = rows // 128                    # 4 rows per partition
    GMAX = 128
    NU16 = s_len // 16                 # 128 16-step units per row
    x = nc.dram_tensor("x", [rows, NCOL], F32, kind="ExternalInput")
    out = nc.dram_tensor("out", [rows, 7], F32, kind="ExternalOutput")
    xv = x.rearrange("(g p) n -> g p n", g=g)

    V = nc.vector
    Gp = nc.gpsimd
    Sc = nc.scalar

    def cross_into(dst, u, v, tmps):
        """dst = u x v per-component (lists of comp views). 9 ops."""
        for i in range(3):
            i1, i2 = (i + 1) % 3, (i + 2) % 3
            V.tensor_tensor(out=tmps[0], in0=u[i1], in1=v[i2], op=OP.mult)
            V.tensor_tensor(out=tmps[1], in0=u[i2], in1=v[i1], op=OP.mult)
            V.tensor_tensor(out=dst[i], in0=tmps[0], in1=tmps[1],
                            op=OP.subtract)

    with TileContext(nc) as tc:
        with (
            tc.tile_pool(name="slab", bufs=1) as slab_pool,
            tc.tile_pool(name="slabh", bufs=2) as slabh_pool,
            tc.tile_pool(name="plane", bufs=1) as plane_pool,
            tc.tile_pool(name="pack", bufs=1) as pack_pool,
            tc.tile_pool(name="upper", bufs=1) as upper_pool,
            tc.tile_pool(name="xtmp", bufs=2) as xtmp_pool,
            tc.tile_pool(name="acc", bufs=1) as acc_pool,
            tc.tile_pool(name="tree", bufs=1) as tree_pool,
        ):
            pshape = [128, g, 3, GMAX]

            def hplane(tg, nm, n=GMAX):
                return plane_pool.tile([128, g, 3, n], BF16, tag=tg,
                                       name=nm)

            def btmp(n):
                return [xtmp_pool.tile([128, g, 3 * GMAX], BF16, tag="vt",
                                       name="vt")[:, :, 0:n]
                        for _ in range(2)]

            # persistent accumulator: [th16|b16|r16] triple at fixed
            # 128-wide segments (bf16 so SA2 writes at 2x and upper
            # level 0 reads at 2x)
            ACC = acc_pool.tile([128, g, 3, 3 * GMAX], BF16, tag="acc3",
                                name="acc3")

            def tri(ap, s=3):
                # [128,g,c,(s n)] -> [128,(g c),s,n] triple-segment view
                return ap.rearrange("p g c (s n) -> p (g c) s n", s=s)

            def sa_level(TIN, SIN, UI, TOUT, off, oth, ob, orr, n_steps,
                         cst, pk):
                """Compose adjacent units of a [th|b|r] triple tile.
                TIN: triple AP [128,g,3,3*SIN] (segments at SIN offsets,
                live width 2*UI each); TOUT: triple AP with segments at
                SOUT offsets; oth/ob/orr: the three live output segment
                views [128,g,3,UI].  One pair-add covers all three
                segments; cst gets the packed [csa|csb] cross via the
                ScalarE-packed pk tiles."""
                U = UI
                thI = TIN[:, :, :, 0:2 * U]
                bI = TIN[:, :, :, SIN:SIN + 2 * U]
                csa, csb = cst[:, :, :, 0:U], cst[:, :, :, U:2 * U]
                PL, PR = pk
                for ci in range(3):
                    Sc.copy(PL[:, :, ci, 0:U], thI[:, :, ci, 0::2])
                    Sc.copy(PL[:, :, ci, U:2 * U], thI[:, :, ci, 0::2])
                    Sc.copy(PR[:, :, ci, 0:U], thI[:, :, ci, 1::2])
                    Sc.copy(PR[:, :, ci, U:2 * U], bI[:, :, ci, 1::2])
                cross_into([cst[:, :, ci, :] for ci in range(3)],
                           [PL[:, :, ci, :] for ci in range(3)],
                           [PR[:, :, ci, :] for ci in range(3)],
                           btmp(2 * U))
                ein = tri(TIN, s=3)[:, :, :, 0:2 * U]
                V.tensor_tensor(out=tri(TOUT, s=3)[:, :, :, off:off + U],
                                in0=ein[:, :, :, 0::2],
                                in1=ein[:, :, :, 1::2], op=OP.add)
                V.scalar_tensor_tensor(out=fuse(oth), in0=fuse(csa),
                                       scalar=DT / 2.0, in1=fuse(oth),
                                       op0=OP.mult, op1=OP.add)
                V.scalar_tensor_tensor(out=fuse(ob), in0=fuse(csb),
                                       scalar=DT, in1=fuse(ob),
                                       op0=OP.mult, op1=OP.add)
                V.scalar_tensor_tensor(out=fuse(orr),
                                       in0=fuse(bI)[:, :, 0::2],
                                       scalar=float(n_steps), in1=fuse(orr),
                                       op0=OP.mult, op1=OP.add)

            s0 = 0
            for si, S_c in enumerate(CHUNKS):
                G = S_c // 4
                U8 = S_c // 8
                U16 = S_c // 16
                u16o = s0 // 16

                sf = slab_pool.tile([128, g, 512 * C], F32, tag="slab",
                                    name=f"slab{si}")
                sh = slabh_pool.tile([128, g, 512 * C], BF16, tag="slabh",
                                     name=f"slabh{si}")
                # DMA phases 0-1 and 2-3 separately so the first casts
                # (and s01) start while the second half is still in
                # flight; ScalarE casts each phase block to bf16 (dense)
                H = 12 * G
                for half in range(2):
                    for gi in range(g):
                        nc.sync.dma_start(
                            out=sf[:, gi, half * H:(half + 1) * H],
                            in_=xv[gi, :, s0 * C + half * H:
                                   s0 * C + (half + 1) * H])
                for ph in range(4):
                    Sc.copy(sh[:, :, ph * 6 * G:(ph + 1) * 6 * G],
                            sf[:, :, ph * 6 * G:(ph + 1) * 6 * G])
                shv = sh[:, :, 0:S_c * C].rearrange(
                    "p g (ph ch n) -> p g ph ch n", ph=4, ch=C)
                W = [shv[:, :, i, 0:3, :] for i in range(4)]
                At = [shv[:, :, i, 3:6, :] for i in range(4)]

                # packed cross operands: Lh = [s01|s01|s23],
                # Rh = [s23|m1|m2], Ch = [c3|cr1|cr2]
                Lh = pack_pool.tile([128, g, 3, 3 * GMAX], BF16, tag="Lh",
                                    name="Lh")
                Rh = pack_pool.tile([128, g, 3, 3 * GMAX], BF16, tag="Rh",
                                    name="Rh")
                Ch = pack_pool.tile([128, g, 3, 3 * GMAX], BF16, tag="Ch",
                                    name="Ch")
                s01 = Lh[:, :, :, 0:G]
                s23 = Lh[:, :, :, 2 * G:3 * G]
                TP = hplane("tp", "tp", n=3 * GMAX)
                th4 = TP[:, :, :, 0:G]
                V.tensor_tensor(out=s01, in0=W[0], in1=W[1], op=OP.add)
                V.tensor_tensor(out=s23, in0=W[2], in1=W[3], op=OP.add)
                for ci in range(3):
                    Sc.copy(Lh[:, :, ci, G:2 * G], s01[:, :, ci, :])
                    Sc.copy(Rh[:, :, ci, 0:G], s23[:, :, ci, :])
                V.tensor_tensor(out=fuse(th4), in0=fuse(s01), in1=fuse(s23),
                                op=OP.add)

                # a-path via pair sums (all bf16 2x, 8 ops):
                #   v1 = A0+A1, v2 = A2+A3, V = v1+v2, t = A0+A2
                #   rho = 3A0+2A1+A2 = 2*v1 + t
                #   m2 = A2+2A3 = v2 + A3,  m1 = 2V-A0 = V + (V-A0)
                # th4/vv/rho4 live in one [th|b|r] triple tile (segments
                # at GMAX offsets) so SA1 pair-adds all three at once
                v1 = hplane("v1", "v1")[:, :, :, 0:G]
                v2 = hplane("rr", "v2")[:, :, :, 0:G]
                tt = hplane("t02", "t02")[:, :, :, 0:G]
                vv = TP[:, :, :, GMAX:GMAX + G]
                rho4 = TP[:, :, :, 2 * GMAX:2 * GMAX + G]
                m1h = Rh[:, :, :, G:2 * G]
                m2h = Rh[:, :, :, 2 * G:3 * G]
                V.tensor_tensor(out=v1, in0=At[0], in1=At[1], op=OP.add)
                V.tensor_tensor(out=v2, in0=At[2], in1=At[3], op=OP.add)
                V.tensor_tensor(out=fuse(vv), in0=fuse(v1), in1=fuse(v2),
                                op=OP.add)
                V.tensor_tensor(out=tt, in0=At[0], in1=At[2], op=OP.add)
                V.scalar_tensor_tensor(out=fuse(rho4), in0=fuse(v1),
                                       scalar=2.0, in1=fuse(tt),
                                       op0=OP.mult, op1=OP.add)
                V.tensor_tensor(out=m2h, in0=v2, in1=At[3], op=OP.add)
                V.tensor_tensor(out=tt, in0=vv, in1=At[0], op=OP.subtract)
                V.tensor_tensor(out=fuse(m1h), in0=fuse(vv), in1=fuse(tt),
                                op=OP.add)

                # packed crosses: [c3|cr1|cr2] = [s01|s01|s23] x
                # [s23|m1|m2] as one 3G-wide cross (9 ops, not 27)
                cross_into([Ch[:, :, ci, 0:3 * G] for ci in range(3)],
                           [Lh[:, :, ci, 0:3 * G] for ci in range(3)],
                           [Rh[:, :, ci, 0:3 * G] for ci in range(3)],
                           btmp(3 * G))
                c3h = Ch[:, :, :, 0:G]
                cr1 = Ch[:, :, :, G:2 * G]
                cr2 = Ch[:, :, :, 2 * G:3 * G]
                V.scalar_tensor_tensor(out=fuse(th4), in0=fuse(c3h),
                                       scalar=DT / 2.0, in1=fuse(th4),
                                       op0=OP.mult, op1=OP.add)
                V.tensor_tensor(out=fuse(cr1), in0=fuse(cr2),
                                in1=fuse(cr1), op=OP.add)
                V.scalar_tensor_tensor(out=fuse(vv), in0=fuse(cr1),
                                       scalar=DT / 2.0, in1=fuse(vv),
                                       op0=OP.mult, op1=OP.add)

                # ---- SA1: n=4 -> 8 (triple tile, packed crosses) ----
                TP8 = hplane("tp8", "tp8", n=192)
                th8 = TP8[:, :, :, 0:U8]
                b8 = TP8[:, :, :, 64:64 + U8]
                r8 = TP8[:, :, :, 128:128 + U8]
                cs1 = hplane("cs", "cs1")
                pl1 = hplane("pl", "pl1")
                pr1 = hplane("pr", "pr1")
                sa_level(TP[:], GMAX, U8, TP8[:], 0, th8, b8, r8, 4,
                         cs1[:, :, :, 0:2 * U8],
                         (pl1[:, :, :, 0:2 * U8], pr1[:, :, :, 0:2 * U8]))

                # ---- SA2: n=8 -> 16, into the bf16 triple accumulator --
                oth = ACC[:, :, :, u16o:u16o + U16]
                ob = ACC[:, :, :, 128 + u16o:128 + u16o + U16]
                orr = ACC[:, :, :, 256 + u16o:256 + u16o + U16]
                cs2 = hplane("cs", "cs2")
                pl2 = hplane("pl", "pl2")
                pr2 = hplane("pr", "pr2")
                sa_level(TP8[:], 64, U16, ACC[:], u16o, oth, ob, orr, 8,
                         cs2[:, :, :, 0:2 * U16],
                         (pl2[:, :, :, 0:2 * U16], pr2[:, :, :, 0:2 * U16]))
                s0 += S_c

            # ---- batched SA levels: 16 -> 32 -> 64 -> 128 -> 256 ----
            # triple tiles throughout; segments at fixed 64 offsets for
            # the intermediate levels, 8 for the final 256-step triple
            cur, cseg = ACC[:], GMAX
            nu = NU16
            n_steps = 16
            for lev in range(4):
                nu //= 2
                if lev < 3:
                    a = lev % 2
                    TU = upper_pool.tile([128, g, 3, 192], F32,
                                         tag=f"u{a}", name=f"u{lev}")[:]
                    oseg = 64
                else:
                    TU = upper_pool.tile([128, g, 3, 24], F32,
                                         tag="t256", name="t256")[:]
                    oseg = 8
                cst = hplane("cs", f"ucs{lev}")
                plu = hplane("pl", f"upl{lev}")
                pru = hplane("pr", f"upr{lev}")
                sa_level(cur, cseg, nu, TU, 0,
                         TU[:, :, :, 0:nu],
                         TU[:, :, :, oseg:oseg + nu],
                         TU[:, :, :, 2 * oseg:2 * oseg + nu], n_steps,
                         cst[:, :, :, 0:2 * nu],
                         (plu[:, :, :, 0:2 * nu], pru[:, :, :, 0:2 * nu]))
                cur, cseg = TU, oseg
                n_steps *= 2

            # ---- convert 256-step units to quaternions ----
            NU = 8
            th256 = cur[:, :, :, 0:NU]
            bd256 = cur[:, :, :, NU:3 * NU].rearrange(
                "p g c (v n) -> p v g c n", v=2)
            K2 = (DT / 2.0) ** 2
            h2 = tree_pool.tile([128, g, NU], F32, tag="h2", name="h2")[:]
            hy = tree_pool.tile([128, g, NU], F32, tag="hy", name="hy")[:]
            hz = tree_pool.tile([128, g, NU], F32, tag="hz", name="hz")[:]
            q256 = tree_pool.tile([128, g, 4, NU], F32, tag="q256",
                                  name="q256")
            thc = [th256[:, :, ci, :] for ci in range(3)]
            V.tensor_tensor(out=h2, in0=thc[0], in1=thc[0], op=OP.mult)
            for ci in (1, 2):
                V.tensor_tensor(out=hy, in0=thc[ci], in1=thc[ci], op=OP.mult)
                V.tensor_tensor(out=h2, in0=h2, in1=hy, op=OP.add)
            V.scalar_tensor_tensor(out=hy, in0=h2, scalar=K2 * K2 / 24.0,
                                   in1=h2, op0=OP.mult, op1=OP.mult)
            V.scalar_tensor_tensor(out=hz, in0=h2, scalar=-K2 / 2.0,
                                   in1=hy, op0=OP.mult, op1=OP.add)
            Sc.activation(q256[:, :, 0, :], hz, AF.Identity, bias=1.0)
            V.scalar_tensor_tensor(out=hy, in0=h2, scalar=K2 * K2 / 120.0,
                                   in1=h2, op0=OP.mult, op1=OP.mult)
            V.scalar_tensor_tensor(out=hz, in0=h2, scalar=-K2 / 6.0,
                                   in1=hy, op0=OP.mult, op1=OP.add)
            Sc.activation(hz, hz, AF.Copy, scale=DT / 2.0, bias=DT / 2.0)
            for ci in range(3):
                V.tensor_tensor(out=q256[:, :, 1 + ci, :],
                                in0=thc[ci], in1=hz, op=OP.mult)

            # ---- quaternion binary tree over 8 units (3 levels) ----
            qt = q256
            bdt = bd256
            n2 = 256.0
            n_lvl = int(math.log2(NU))
            for lvl in range(1, n_lvl + 1):
                n = NU >> lvl
                nqt = tree_pool.tile([128, g, 4, n], F32, tag=f"nq{lvl}",
                                     name=f"nq{lvl}")
                nbd = tree_pool.tile([128, 2, g, 3, n], F32, tag=f"nbd{lvl}",
                                     name=f"nbd{lvl}")
                P = [tree_pool.tile([128, g, 4, n], F32, tag=f"P{a}{lvl}",
                                    name=f"P{a}")[:] for a in range(4)]
                tcc = tree_pool.tile([128, 2, g, 3, n], F32,
                                     tag=f"tcc{lvl}", name="tcc")
                tcq = tree_pool.tile([128, 2, g, 3, n], F32,
                                     tag=f"tcq{lvl}", name="tcq")
                tcw = tree_pool.tile([128, 2, g, 3, n], F32,
                                     tag=f"tcw{lvl}", name="tcw")
                tms = [tree_pool.tile([128, 2, g, n], F32,
                                      tag=f"tm{i}{lvl}", name="tms")[:]
                       for i in range(2)]

                q2all = qt[:, :, :, 1::2]
                for a in range(4):
                    q1a = qt[:, :, a, 0::2].unsqueeze(2).broadcast_to(
                        [128, g, 4, n])
                    V.tensor_tensor(out=P[a], in0=q1a, in1=q2all, op=OP.mult)
                for c in range(4):
                    idx = QIDX[c]
                    sgn = QSGN[c]
                    acc = nqt[:, :, c, :]
                    V.tensor_tensor(
                        out=acc, in0=P[0][:, :, idx[0], :],
                        in1=P[1][:, :, idx[1], :],
                        op=OP.add if sgn[1] > 0 else OP.subtract)
                    for t in (2, 3):
                        V.tensor_tensor(
                            out=acc, in0=acc, in1=P[t][:, :, idx[t], :],
                            op=OP.add if sgn[t] > 0 else OP.subtract)

                def qb(comp):
                    return qt[:, :, comp, 0::2].unsqueeze(1).broadcast_to(
                        [128, 2, g, n])

                def bd2(comp):
                    return bdt[:, :, :, comp, 1::2]

                # tcq and the nbd pair-add are independent of the serial
                # tcc->tcw chain, so GpSimd takes those two pieces.
                for i in range(3):
                    i1, i2 = (i + 1) % 3, (i + 2) % 3
                    V.tensor_tensor(out=tms[0], in0=qb(1 + i1),
                                    in1=bd2(i2), op=OP.mult)
                    V.tensor_tensor(out=tms[1], in0=qb(1 + i2),
                                    in1=bd2(i1), op=OP.mult)
                    V.tensor_tensor(out=tcc[:, :, :, i, :], in0=tms[0],
                                    in1=tms[1], op=OP.subtract)
                for i in range(3):
                    V.tensor_tensor(out=tcq[:, :, :, i, :], in0=qb(0),
                                    in1=bd2(i), op=OP.mult)
                V.tensor_tensor(out=tcc[:].opt(), in0=tcc[:].opt(),
                                in1=tcq[:].opt(), op=OP.add)
                for i in range(3):
                    i1, i2 = (i + 1) % 3, (i + 2) % 3
                    V.tensor_tensor(out=tms[0], in0=qb(1 + i1),
                                    in1=tcc[:, :, :, i2, :], op=OP.mult)
                    V.tensor_tensor(out=tms[1], in0=qb(1 + i2),
                                    in1=tcc[:, :, :, i1, :], op=OP.mult)
                    V.tensor_tensor(out=tcw[:, :, :, i, :], in0=tms[0],
                                    in1=tms[1], op=OP.subtract)
                V.tensor_tensor(out=nbd[:].opt(),
                                in0=bdt[:, :, :, :, 0::2].opt(),
                                in1=bdt[:, :, :, :, 1::2].opt(), op=OP.add)
                V.scalar_tensor_tensor(out=nbd[:].opt(), in0=tcw[:].opt(),
                                       scalar=2.0, in1=nbd[:].opt(),
                                       op0=OP.mult, op1=OP.add)
                V.scalar_tensor_tensor(
                    out=nbd[:, 1].rearrange("p g c n -> p (g c) n"),
                    in0=bdt[:, 0, :, :, 0::2].rearrange(
                        "p g c n -> p (g c) n"),
                    scalar=n2,
                    in1=nbd[:, 1].rearrange("p g c n -> p (g c) n"),
                    op0=OP.mult, op1=OP.add)

                qt, bdt = nqt, nbd
                n2 *= 2.0

            cur_q = [qt[:, :, ci, :] for ci in range(4)]
            cur_b = [bdt[:, 0, :, ci, :] for ci in range(3)]
            cur_d = [bdt[:, 1, :, ci, :] for ci in range(3)]

            # ---- finalize ----
            out_t = tree_pool.tile([128, g, 7], F32, tag="outt",
                                   name="outt")[:]
            tf = tree_pool.tile([128, g, 3], F32, tag="tf", name="tf")[:]
            for i in range(3):
                V.scalar_tensor_tensor(out=tf[:, :, i:i + 1], in0=cur_b[i],
                                       scalar=1.5, in1=cur_d[i],
                                       op0=OP.mult, op1=OP.add)
            V.tensor_scalar(out=out_t[:, :, 0:3], in0=tf,
                            scalar1=DT * DT, scalar2=None, op0=OP.mult)
            sg = tree_pool.tile([128, g, 1], F32, tag="sg", name="sg")[:]
            V.tensor_scalar(out=sg, in0=cur_q[0], scalar1=0.0, scalar2=2.0,
                            op0=OP.is_ge, op1=OP.mult)
            V.tensor_scalar(out=sg, in0=sg, scalar1=-1.0, scalar2=None,
                            op0=OP.add)
            V.tensor_tensor(out=out_t[:, :, 3:7],
                            in0=qt[:, :, :, 0],
                            in1=sg.broadcast_to([128, g, 4]), op=OP.mult)

            ov = out.rearrange("(g p) c -> p g c", g=g)
            nc.sync.dma_start(out=ov, in_=out_t)

    nc.compile()
    return nc


_NC_CACHE = {}
LAST_RESULTS = None


def _ensure_profiling_hooks():
    """Best-effort: provide the antenv.axon_hooks shim + skip S3 upload so
    trace=True works in this stripped container. No-op on failure."""
    import sys
    import types
    try:
        if "antenv.axon_hooks" not in sys.modules:
            from trn_agent_boot.trn_boot import _ntff_profile_via_ctypes
            hook = _ntff_profile_via_ctypes("/opt/axon/libaxon_pjrt.so")
            mod = types.ModuleType("antenv.axon_hooks")
            mod._hook = hook
            mod.get_axon_ntff_profile_hook = lambda: mod._hook
            mod.set_axon_ntff_profile_hook = lambda h: setattr(mod, "_hook", h)
            sys.modules["antenv.axon_hooks"] = mod
        import concourse.bass_utils as bu
        bu.upload_artifacts = lambda tmpdir: tmpdir
    except Exception as e:  # pragma: no cover
        print(f"profiling hook setup failed ({e}); tracing may be skipped")


def kernel(input_seq: np.ndarray) -> np.ndarray:
    from concourse.bass_utils import run_bass_kernel_spmd

    global LAST_RESULTS
    input_seq = np.ascontiguousarray(np.asarray(input_seq, dtype=np.float32))
    assert input_seq.shape == (B_FULL, S_FULL, C), input_seq.shape
    xp = host_permute(input_seq)

    if "nc" not in _NC_CACHE:
        _NC_CACHE["nc"] = build_nc()
    nc = _NC_CACHE["nc"]

    in_maps = [{"x": xp[i * R:(i + 1) * R]} for i in range(N_CORES)]
    trace = os.environ.get("BASS_KERNEL_TRACE", "0") == "1"
    if trace:
        _ensure_profiling_hooks()
    try:
        res = run_bass_kernel_spmd(nc, in_maps, core_ids=list(range(N_CORES)),
                                   trace=trace)
    except Exception:
        # transient device wedge (NRT_EXEC_UNIT_UNRECOVERABLE) recovers on
        # a clean re-run; retry once
        res = run_bass_kernel_spmd(nc, in_maps, core_ids=list(range(N_CORES)),
                                   trace=trace)
    LAST_RESULTS = res
    return np.concatenate([r["out"] for r in res.results], axis=0)
